# revision 3
# baseline (speedup 1.0000x reference)
"""Trainium2 Bass kernel: pre-LN multi-head attention block (B=8, L=1024,
D=1024, H=16, dk=dv=64), data-parallel over batch across 8 NeuronCores.

v2 design (all-bf16 PE path):
  - k, v pre-cast to bf16 on host; q stays fp32 (LN + residual precision).
  - pre-LN gamma/beta folded into Wq on host: Q = xhat @ wq' + bq, with the
    bias added during the PSUM->SBUF evacuation (per-partition AP scalar).
  - x^T built by PE transposes in bf16 (1 cyc/row), 8 blocks per psum tile,
    single strided DVE evac per token tile.
  - S^T per head via 64-partition-offset matmul operands (no zero-padding).
  - exp on ACT reads a [128,1024] psum tile (2 banks / 2 matmul groups).
  - PV with ones-augmented V (sumexp rides along as psum row 64); lane-64
    reciprocal + 1-partition matmul broadcast; no sumexp DMAs.
  - output projection flipped token-major (stationary O^T chunks, moving Wo
    rows): no output transposes; residual add + LN + store pipelined per
    token tile.
"""

import numpy as np
import ml_dtypes

import concourse.bass as bass
import concourse.mybir as mybir
import concourse.tile as tile
from concourse import bacc

P = 128
L = 1024          # tokens per batch element
D = 1024          # model dim
H = 16            # heads
HD = 64           # head dim
E = HD + 1        # head dim + sumexp column
NC = D // P       # 8 feature chunks
NT = L // P       # 8 token chunks
NQ = 2            # 512-wide halves of the moving/free dimension
QH = 512
EPS = 1e-6

FP32 = mybir.dt.float32
BF16 = mybir.dt.bfloat16
FP32R = mybir.dt.float32r
OP = mybir.AluOpType
AF = mybir.ActivationFunctionType


def _emit_ln_stats(nc, pool, x, scratch, eps_t):
    """Return (rstd, neg_mu_rstd) per-partition [P,1] APs for LN of x."""
    st = pool.tile([P, 8], FP32, tag="lnst", bufs=4, name="lnst")
    nc.scalar.activation(scratch, x, AF.Copy, accum_out=st[:, 0:1])
    nc.scalar.activation(scratch, x, AF.Square, accum_out=st[:, 1:2])
    nc.vector.tensor_scalar_mul(st[:, 2:3], st[:, 0:1], 1.0 / D)     # mu
    nc.vector.tensor_tensor(st[:, 3:4], st[:, 2:3], st[:, 2:3], OP.mult)
    nc.vector.tensor_scalar_mul(st[:, 4:5], st[:, 1:2], 1.0 / D)     # E[x^2]
    nc.vector.tensor_tensor(st[:, 4:5], st[:, 4:5], st[:, 3:4], OP.subtract)
    nc.scalar.activation(st[:, 5:6], st[:, 4:5], AF.Sqrt, bias=eps_t)
    nc.vector.reciprocal(st[:, 6:7], st[:, 5:6])                     # rstd
    nc.vector.tensor_tensor(st[:, 7:8], st[:, 2:3], st[:, 6:7], OP.mult)
    nc.vector.tensor_scalar_mul(st[:, 7:8], st[:, 7:8], -1.0)        # -mu*rstd
    return st[:, 6:7], st[:, 7:8]


def build_bass():
    nc = bacc.Bacc("TRN2", target_bir_lowering=False, debug=False)

    q_d = nc.dram_tensor("q", [L, D], FP32, kind="ExternalInput")
    kb_d = nc.dram_tensor("kb", [L, D], BF16, kind="ExternalInput")
    vb_d = nc.dram_tensor("vb", [L, D], BF16, kind="ExternalInput")
    wq_d = nc.dram_tensor("wq", [D, D], BF16, kind="ExternalInput")
    wk_d = nc.dram_tensor("wk", [D, D], BF16, kind="ExternalInput")
    wv_d = nc.dram_tensor("wv", [D, D], BF16, kind="ExternalInput")
    wo_d = nc.dram_tensor("wo", [D, D], BF16, kind="ExternalInput")
    bq_d = nc.dram_tensor("bq", [P, NC], FP32, kind="ExternalInput")
    gb_d = nc.dram_tensor("gb", [P, D], FP32, kind="ExternalInput")
    bb_d = nc.dram_tensor("bb", [P, D], FP32, kind="ExternalInput")
    id_d = nc.dram_tensor("ident", [P, P], BF16, kind="ExternalInput")
    on_d = nc.dram_tensor("ones64", [P, HD], FP32R, kind="ExternalInput")
    ep_d = nc.dram_tensor("epsc", [P, 1], FP32, kind="ExternalInput")
    vo_d = nc.dram_tensor("vone", [P, H * E], BF16, kind="ExternalInput")
    out_d = nc.dram_tensor("out", [L, D], FP32, kind="ExternalOutput")

    with tile.TileContext(nc) as tc:
        with tc.tile_pool(name="persist", bufs=1) as pp:
            ident = pp.tile([P, P], BF16, name="ident")
            eps_t = pp.tile([P, 1], FP32, name="eps_t")
            ones64 = pp.tile([P, HD], FP32R, name="ones64")
            bq_t = pp.tile([P, NC], FP32, name="bq_t")
            KT = pp.tile([P, NC, L], BF16, name="KT")
            QT = pp.tile([P, NC, L], BF16, name="QT")
            Vaug = pp.tile([P, NT, H * E], BF16, name="Vaug")
            OT = [pp.tile([P, L], BF16, name=f"ot{j}") for j in range(H // 2)]

            nc.sync.dma_start(ident, id_d[:])
            nc.sync.dma_start(eps_t, ep_d[:])
            nc.sync.dma_start(ones64, on_d[:])
            nc.sync.dma_start(bq_t, bq_d[:])

            # ---------------- QKV phase ----------------
            with (
                tc.tile_pool(name="qkv", bufs=1) as qp,
                tc.tile_pool(name="psA", bufs=1, space="PSUM") as psA,
            ):
                def load_w(dram, nm):
                    tiles = []
                    for i in range(NC):
                        wt = qp.tile([P, D], BF16, tag=f"w{nm}", bufs=NC,
                                     name=f"w{nm}{i}")
                        nc.sync.dma_start(wt, dram[i * P:(i + 1) * P, :])
                        tiles.append(wt)
                    return tiles

                wk_t = load_w(wk_d, "k")
                for t in range(NT):
                    nc.sync.dma_start(Vaug[:, t, :], vo_d[:])

                def transpose_tile(dst, x, t):
                    """dst[:, c, t*128:+128] = x[:, c*128:+128]^T for all c."""
                    pt = psA.tile([P, D], BF16, tag="tr", bufs=2, name="ps_tr")
                    for c in range(NC):
                        nc.tensor.transpose(
                            pt[:, c * P:(c + 1) * P],
                            x[:, c * P:(c + 1) * P], ident)
                    nc.vector.tensor_copy(
                        dst[:, :, t * P:(t + 1) * P],
                        pt.rearrange("p (c x) -> p c x", x=P))

                def xT_tile():
                    return qp.tile([P, NC, L], BF16, tag="xT", bufs=2,
                                   name="xT")

                # ---- k -> kT -> K-proj ----
                kT = xT_tile()
                for t in range(NT):
                    x = qp.tile([P, D], BF16, tag="kin", bufs=4, name="k_in")
                    nc.sync.dma_start(x, kb_d[t * P:(t + 1) * P, :])
                    transpose_tile(kT, x, t)

                def proj_feat(w_tiles, src, dst, bias_col=None):
                    for m in range(NC):
                        ps = psA.tile([P, L], FP32, tag="pj", bufs=2,
                                      name="ps_pj")
                        for n in range(NQ):
                            for i in range(NC):
                                nc.tensor.matmul(
                                    ps[:, n * QH:(n + 1) * QH],
                                    w_tiles[i][:, m * P:(m + 1) * P],
                                    src[:, i, n * QH:(n + 1) * QH],
                                    start=(i == 0), stop=(i == NC - 1))
                        if bias_col is None:
                            nc.vector.tensor_copy(dst[:, m, :], ps)
                        else:
                            nc.vector.tensor_scalar_add(
                                dst[:, m, :], ps, bias_col[:, m:m + 1])

                proj_feat(wk_t, kT, KT)
                wv_t = load_w(wv_d, "v")

                # ---- v -> vT -> V-proj (token-major, into Vaug) ----
                vT = xT_tile()
                for t in range(NT):
                    x = qp.tile([P, D], BF16, tag="kin", bufs=4, name="v_in")
                    nc.sync.dma_start(x, vb_d[t * P:(t + 1) * P, :])
                    transpose_tile(vT, x, t)
                wq_t = load_w(wq_d, "q")
                for t in range(NT):
                    ps = psA.tile([P, L], FP32, tag="pj", bufs=2, name="ps_v")
                    for n in range(NQ):
                        for i in range(NC):
                            nc.tensor.matmul(
                                ps[:, n * QH:(n + 1) * QH],
                                vT[:, i, t * P:(t + 1) * P],
                                wv_t[i][:, n * QH:(n + 1) * QH],
                                start=(i == 0), stop=(i == NC - 1))
                    dst = Vaug[:, t, :].rearrange("p (h e) -> p h e", e=E)
                    nc.vector.tensor_copy(
                        dst[:, :, 0:HD],
                        ps.rearrange("p (h x) -> p h x", x=HD))

                # ---- q -> LN -> qnT -> Q-proj (bias folded) ----
                qnT = xT_tile()
                for t in range(NT):
                    x = qp.tile([P, D], FP32, tag="qin", bufs=3, name="q_in")
                    nc.sync.dma_start(x, q_d[t * P:(t + 1) * P, :])
                    y = qp.tile([P, D], BF16, tag="qn", bufs=3, name="qn")
                    rstd, nmr = _emit_ln_stats(nc, qp, x, y, eps_t)
                    nc.scalar.activation(y, x, AF.Identity, bias=nmr,
                                         scale=rstd)
                    transpose_tile(qnT, y, t)
                proj_feat(wq_t, qnT, QT, bias_col=bq_t)

            # ---------------- out-phase inputs (emit DMAs early) ----------
            with tc.tile_pool(name="fin", bufs=1) as fp:
                gamma_bc = fp.tile([P, D], FP32, name="gamma_bc")
                beta_bc = fp.tile([P, D], FP32, name="beta_bc")
                nc.sync.dma_start(gamma_bc, gb_d[:])
                nc.sync.dma_start(beta_bc, bb_d[:])
                wo_t = []
                for j in range(NC):
                    wt = fp.tile([P, D], BF16, tag="wo", bufs=NC,
                                 name=f"wo{j}")
                    nc.sync.dma_start(wt, wo_d[j * P:(j + 1) * P, :])
                    wo_t.append(wt)
                res = []
                for t in range(NT):
                    rt = fp.tile([P, D], FP32, tag="res", bufs=NT,
                                 name=f"res{t}")
                    nc.sync.dma_start(rt, q_d[t * P:(t + 1) * P, :])
                    res.append(rt)

                # ---------------- attention ----------------
                with (
                    tc.tile_pool(name="att", bufs=1) as ap,
                    tc.tile_pool(name="psS", bufs=2, space="PSUM") as psS,
                    tc.tile_pool(name="psO", bufs=3, space="PSUM") as psO,
                    tc.tile_pool(name="psB", bufs=1, space="PSUM") as psB,
                ):
                    for h in range(H):
                        c, half = h // 2, h % 2
                        hs = slice(half * HD, half * HD + HD)
                        PT = ap.tile([P, NC, L], BF16, tag="pt", bufs=2,
                                     name="pt")
                        for i in range(NT):
                            ks = slice(i * P, (i + 1) * P)
                            ss = psS.tile([P, L], FP32, tag="s", name="ps_s")
                            for n in range(NQ):
                                nc.tensor.matmul(
                                    ss[:, n * QH:(n + 1) * QH],
                                    KT[hs, c, ks],
                                    QT[hs, c, n * QH:(n + 1) * QH],
                                    start=True, stop=True)
                            nc.scalar.activation(PT[:, i, :], ss, AF.Exp)
                        rec = ap.tile([E, L], FP32R, tag="rec", bufs=2,
                                      name="rec")
                        otmp = None
                        if half == 1:
                            otmp = ap.tile([HD, L], BF16, tag="otmp", bufs=2,
                                           name="otmp")
                        for n in range(NQ):
                            ns = slice(n * QH, (n + 1) * QH)
                            po = psO.tile([E, QH], FP32, tag="o", name="ps_o")
                            for i in range(NT):
                                nc.tensor.matmul(
                                    po, Vaug[:, i, h * E:(h + 1) * E],
                                    PT[:, i, ns],
                                    start=(i == 0), stop=(i == NT - 1))
                            with nc.allow_low_precision(
                                    reason="softmax recip"):
                                nc.vector.reciprocal(rec[HD:E, ns],
                                                     po[HD:E, :])
                            pb = psB.tile([HD, QH], FP32, tag="b",
                                          name="ps_b")
                            nc.tensor.matmul(pb, ones64[HD:E, :],
                                             rec[HD:E, ns],
                                             start=True, stop=True)
                            pbs = ap.tile([HD, QH], FP32, tag="pbs", bufs=2,
                                          name="pbs")
                            nc.vector.tensor_copy(pbs, pb)
                            if half == 0:
                                nc.vector.tensor_tensor(
                                    OT[c][0:HD, ns], po[0:HD, :], pbs,
                                    OP.mult)
                            else:
                                nc.vector.tensor_tensor(
                                    otmp[:, ns], po[0:HD, :], pbs, OP.mult)
                        if half == 1:
                            nc.sync.dma_start(OT[c][HD:P, :], otmp)

                # ------------- output projection + residual + LN ---------
                with tc.tile_pool(name="psW", bufs=2, space="PSUM") as psW:
                    for t in range(NT):
                        ps = psW.tile([P, D], FP32, tag="w", name="ps_w")
                        for n in range(NQ):
                            for j in range(NC):
                                nc.tensor.matmul(
                                    ps[:, n * QH:(n + 1) * QH],
                                    OT[j][:, t * P:(t + 1) * P],
                                    wo_t[j][:, n * QH:(n + 1) * QH],
                                    start=(j == 0), stop=(j == NC - 1))
                        u = fp.tile([P, D], FP32, tag="u", bufs=2, name="u")
                        nc.vector.tensor_tensor(u, ps, res[t], OP.add)
                        y = fp.tile([P, D], FP32, tag="y", bufs=2, name="y")
                        rstd, nmr = _emit_ln_stats(nc, fp, u, y, eps_t)
                        nc.scalar.activation(y, u, AF.Identity, bias=nmr,
                                             scale=rstd)
                        nc.vector.tensor_tensor(y, y, gamma_bc, OP.mult)
                        nc.vector.tensor_tensor(y, y, beta_bc, OP.add)
                        nc.sync.dma_start(out_d[t * P:(t + 1) * P, :], y)

    nc.compile()
    return nc


_CACHE = {}


def _get_nc():
    if "nc" not in _CACHE:
        _CACHE["nc"] = build_bass()
    return _CACHE["nc"]


def make_in_maps(q, k, v, Wq, Wk, Wv, Wo, gamma, beta):
    q = np.asarray(q, np.float32)
    kb = np.asarray(k, np.float32).astype(ml_dtypes.bfloat16)
    vb = np.asarray(v, np.float32).astype(ml_dtypes.bfloat16)
    gamma = np.asarray(gamma, np.float32)
    beta = np.asarray(beta, np.float32)
    Wq = np.asarray(Wq, np.float32)
    # fold pre-LN gamma/beta and the 1/sqrt(dk)=0.125 scale into Wq
    wq = (0.125 * gamma[:, None] * Wq).astype(ml_dtypes.bfloat16)
    bq = (0.125 * (beta @ Wq)).astype(np.float32)           # [D]
    bq_t = np.ascontiguousarray(bq.reshape(NC, P).T)        # [P, NC]
    wk = np.asarray(Wk, np.float32).astype(ml_dtypes.bfloat16)
    wv = np.asarray(Wv, np.float32).astype(ml_dtypes.bfloat16)
    wo = np.asarray(Wo, np.float32).astype(ml_dtypes.bfloat16)
    gb = np.ascontiguousarray(np.tile(gamma[None, :], (P, 1)))
    bb = np.ascontiguousarray(np.tile(beta[None, :], (P, 1)))
    ident = np.eye(P, dtype=np.float32).astype(ml_dtypes.bfloat16)
    ones64 = np.ones((P, HD), np.float32)
    epsc = np.full((P, 1), EPS, np.float32)
    vone = np.ones((P, H * E), ml_dtypes.bfloat16)
    B = q.shape[0]
    return [
        {
            "q": np.ascontiguousarray(q[b]),
            "kb": np.ascontiguousarray(kb[b]),
            "vb": np.ascontiguousarray(vb[b]),
            "wq": wq, "wk": wk, "wv": wv, "wo": wo, "bq": bq_t,
            "gb": gb, "bb": bb, "ident": ident, "ones64": ones64,
            "epsc": epsc, "vone": vone,
        }
        for b in range(B)
    ]


def kernel(q, k, v, Wq, Wk, Wv, Wo, gamma, beta, trace=False):
    from concourse.bass_utils import run_bass_kernel_spmd

    nc = _get_nc()
    in_maps = make_in_maps(q, k, v, Wq, Wk, Wv, Wo, gamma, beta)
    res = run_bass_kernel_spmd(nc, in_maps, core_ids=list(range(len(in_maps))),
                               trace=trace)
    out = np.stack([r["out"] for r in res.results], axis=0)
    if trace:
        return out, res
    return out


# revision 13
# speedup vs baseline: 1.2203x; 1.2203x over previous
"""Trainium2 Bass kernel: pre-LN multi-head attention block (B=8, L=1024,
D=1024, H=16, dk=dv=64), data-parallel over batch across 8 NeuronCores.

v2 design (all-bf16 PE path):
  - k, v pre-cast to bf16 on host; q stays fp32 (LN + residual precision).
  - pre-LN gamma/beta folded into Wq on host: Q = xhat @ wq' + bq, with the
    bias added during the PSUM->SBUF evacuation (per-partition AP scalar).
  - x^T built by PE transposes in bf16 (1 cyc/row), 8 blocks per psum tile,
    single strided DVE evac per token tile.
  - S^T per head via 64-partition-offset matmul operands (no zero-padding).
  - exp on ACT reads a [128,1024] psum tile (2 banks / 2 matmul groups).
  - PV with ones-augmented V (sumexp rides along as psum row 64); lane-64
    reciprocal + 1-partition matmul broadcast; no sumexp DMAs.
  - output projection flipped token-major (stationary O^T chunks, moving Wo
    rows): no output transposes; residual add + LN + store pipelined per
    token tile.
"""

import numpy as np
import ml_dtypes

import concourse.bass as bass
import concourse.mybir as mybir
import concourse.tile as tile
from concourse import bacc
from concourse.dve_ops import RECIP_APPROX_FAST_CONSTS, RECIPROCAL_APPROX_FAST

P = 128
L = 1024          # tokens per batch element
D = 1024          # model dim
H = 16            # heads
HD = 64           # head dim
E = HD + 1        # head dim + sumexp column
NC = D // P       # 8 feature chunks
NT = L // P       # 8 token chunks
NQ = 2            # 512-wide halves of the moving/free dimension
QH = 512
EPS = 1e-6

FP32 = mybir.dt.float32
BF16 = mybir.dt.bfloat16
FP32R = mybir.dt.float32r
OP = mybir.AluOpType
AF = mybir.ActivationFunctionType


def _emit_ln_stats(nc, pool, x, scratch, eps_t):
    """Return (rstd, neg_mu_rstd) per-partition [P,1] APs for LN of x."""
    st = pool.tile([P, 8], FP32, tag="lnst", bufs=4, name="lnst")
    nc.scalar.activation(scratch, x, AF.Copy, accum_out=st[:, 0:1])
    nc.scalar.activation(scratch, x, AF.Square, accum_out=st[:, 1:2])
    nc.vector.tensor_scalar_mul(st[:, 2:3], st[:, 0:1], 1.0 / D)     # mu
    nc.vector.tensor_tensor(st[:, 3:4], st[:, 2:3], st[:, 2:3], OP.mult)
    nc.vector.tensor_scalar_mul(st[:, 4:5], st[:, 1:2], 1.0 / D)     # E[x^2]
    nc.vector.tensor_tensor(st[:, 4:5], st[:, 4:5], st[:, 3:4], OP.subtract)
    nc.scalar.activation(st[:, 5:6], st[:, 4:5], AF.Sqrt, bias=eps_t)
    nc.vector.reciprocal(st[:, 6:7], st[:, 5:6])                     # rstd
    nc.vector.tensor_tensor(st[:, 7:8], st[:, 2:3], st[:, 6:7], OP.mult)
    nc.vector.tensor_scalar_mul(st[:, 7:8], st[:, 7:8], -1.0)        # -mu*rstd
    return st[:, 6:7], st[:, 7:8]


def build_bass():
    nc = bacc.Bacc("TRN2", target_bir_lowering=False, debug=False)

    q_d = nc.dram_tensor("q", [L, D], FP32, kind="ExternalInput")
    kb_d = nc.dram_tensor("kb", [L, D], BF16, kind="ExternalInput")
    vb_d = nc.dram_tensor("vb", [L, D], BF16, kind="ExternalInput")
    wq_d = nc.dram_tensor("wq", [D, D], BF16, kind="ExternalInput")
    wk_d = nc.dram_tensor("wk", [D, D], BF16, kind="ExternalInput")
    wv_d = nc.dram_tensor("wv", [D, D], BF16, kind="ExternalInput")
    wo_d = nc.dram_tensor("wo", [D, D], BF16, kind="ExternalInput")
    bq_d = nc.dram_tensor("bq", [P, NC], FP32, kind="ExternalInput")
    gb_d = nc.dram_tensor("gb", [P, D], FP32, kind="ExternalInput")
    bb_d = nc.dram_tensor("bb", [P, D], FP32, kind="ExternalInput")
    id_d = nc.dram_tensor("ident", [P, P], BF16, kind="ExternalInput")
    on_d = nc.dram_tensor("ones64", [P, HD], FP32R, kind="ExternalInput")
    ep_d = nc.dram_tensor("epsc", [P, 1], FP32, kind="ExternalInput")
    vo_d = nc.dram_tensor("vone", [P, H * E], BF16, kind="ExternalInput")
    out_d = nc.dram_tensor("out", [L, D], FP32, kind="ExternalOutput")

    with tile.TileContext(nc) as tc:
        with tc.tile_pool(name="persist", bufs=1) as pp:
            ident = pp.tile([P, P], BF16, name="ident")
            eps_t = pp.tile([P, 1], FP32, name="eps_t")
            ones64 = pp.tile([P, HD], FP32R, name="ones64")
            bq_t = pp.tile([P, NC], FP32, name="bq_t")
            KT = pp.tile([P, NC, L], BF16, name="KT")
            QT = pp.tile([P, NC, L], BF16, name="QT")
            Vaug = pp.tile([P, NT, H * E], BF16, name="Vaug")
            OT = [pp.tile([P, L], BF16, name=f"ot{j}") for j in range(H // 2)]

            nc.sync.dma_start(ident, id_d[:])
            nc.sync.dma_start(eps_t, ep_d[:])
            nc.sync.dma_start(ones64, on_d[:])
            nc.sync.dma_start(bq_t, bq_d[:])

            # ---------------- QKV phase ----------------
            with (
                tc.tile_pool(name="qkv", bufs=1) as qp,
                tc.tile_pool(name="psA", bufs=1, space="PSUM") as psA,
            ):
                def load_w(dram, nm):
                    tiles = []
                    for i in range(NC):
                        wt = qp.tile([P, D], BF16, tag=f"w{nm}", bufs=NC,
                                     name=f"w{nm}{i}")
                        nc.sync.dma_start(wt, dram[i * P:(i + 1) * P, :])
                        tiles.append(wt)
                    return tiles

                def transpose_tile(dst, x, t):
                    """dst[:, c, t*128:+128] = x[:, c*128:+128]^T for all c."""
                    pt = psA.tile([P, D], BF16, tag="tr", bufs=2, name="ps_tr")
                    for c in range(NC):
                        nc.tensor.transpose(
                            pt[:, c * P:(c + 1) * P],
                            x[:, c * P:(c + 1) * P], ident)
                    nc.vector.tensor_copy(
                        dst[:, :, t * P:(t + 1) * P],
                        pt.rearrange("p (c x) -> p c x", x=P))

                def xT_tile():
                    return qp.tile([P, NC, L], BF16, tag="xT", bufs=2,
                                   name="xT")

                # ---- k -> kT -> K-proj ----
                # kb DMAs go first on the sync queue; tile 0 is split into
                # column chunks across DMA queues so the first transpose can
                # start within ~2us.
                kT = xT_tile()
                kin = []
                for t in range(NT):
                    x = qp.tile([P, D], BF16, tag="kin", bufs=NT, name="k_in")
                    if t == 0:
                        for c in range(NC):
                            cs = slice(c * P, (c + 1) * P)
                            nc.sync.dma_start(x[:, cs], kb_d[0:P, cs])
                    else:
                        nc.sync.dma_start(x, kb_d[t * P:(t + 1) * P, :])
                    kin.append(x)
                wk_t = load_w(wk_d, "k")
                for t in range(NT):
                    transpose_tile(kT, kin[t], t)

                def proj_feat(w_tiles, src, dst, bias_col=None):
                    for m in range(NC):
                        ps = psA.tile([P, L], FP32, tag="pj", bufs=2,
                                      name="ps_pj")
                        for n in range(NQ):
                            for i in range(NC):
                                nc.tensor.matmul(
                                    ps[:, n * QH:(n + 1) * QH],
                                    w_tiles[i][:, m * P:(m + 1) * P],
                                    src[:, i, n * QH:(n + 1) * QH],
                                    start=(i == 0), stop=(i == NC - 1))
                        if bias_col is None:
                            nc.vector.tensor_copy(dst[:, m, :], ps)
                        else:
                            nc.vector.tensor_scalar_add(
                                dst[:, m, :], ps, bias_col[:, m:m + 1])

                # prefetch v tiles + ones columns + Wv while K-proj runs
                vin = []
                for t in range(NT):
                    x = qp.tile([P, D], BF16, tag="vin", bufs=NT, name="v_in")
                    nc.sync.dma_start(x, vb_d[t * P:(t + 1) * P, :])
                    vin.append(x)
                for t in range(NT):
                    nc.sync.dma_start(Vaug[:, t, :], vo_d[:])
                wv_t = load_w(wv_d, "v")

                proj_feat(wk_t, kT, KT)

                # ---- v -> vT -> V-proj (token-major, into Vaug) ----
                vT = xT_tile()
                for t in range(NT):
                    transpose_tile(vT, vin[t], t)
                wq_t = load_w(wq_d, "q")
                for t in range(NT):
                    ps = psA.tile([P, L], FP32, tag="pj", bufs=2, name="ps_v")
                    for n in range(NQ):
                        for i in range(NC):
                            nc.tensor.matmul(
                                ps[:, n * QH:(n + 1) * QH],
                                vT[:, i, t * P:(t + 1) * P],
                                wv_t[i][:, n * QH:(n + 1) * QH],
                                start=(i == 0), stop=(i == NC - 1))
                    dst = Vaug[:, t, :].rearrange("p (h e) -> p h e", e=E)
                    nc.vector.tensor_copy(
                        dst[:, :, 0:HD],
                        ps.rearrange("p (h x) -> p h x", x=HD))

                # ---- q -> LN -> qnT -> Q-proj (bias folded) ----
                qnT = xT_tile()
                for t in range(NT):
                    x = qp.tile([P, D], FP32, tag="qin", bufs=3, name="q_in")
                    nc.sync.dma_start(x, q_d[t * P:(t + 1) * P, :])
                    y = qp.tile([P, D], BF16, tag="qn", bufs=3, name="qn")
                    rstd, nmr = _emit_ln_stats(nc, qp, x, y, eps_t)
                    nc.scalar.activation(y, x, AF.Identity, bias=nmr,
                                         scale=rstd)
                    transpose_tile(qnT, y, t)
                proj_feat(wq_t, qnT, QT, bias_col=bq_t)

            # ---------------- out-phase inputs (emit DMAs early) ----------
            with tc.tile_pool(name="fin", bufs=1) as fp:
                gamma_bc = fp.tile([P, D], FP32, name="gamma_bc")
                beta_bc = fp.tile([P, D], FP32, name="beta_bc")
                nc.sync.dma_start(gamma_bc, gb_d[:])
                nc.sync.dma_start(beta_bc, bb_d[:])
                wo_t = []
                for j in range(NC):
                    wt = fp.tile([P, D], BF16, tag="wo", bufs=NC,
                                 name=f"wo{j}")
                    nc.sync.dma_start(wt, wo_d[j * P:(j + 1) * P, :])
                    wo_t.append(wt)
                res = []
                for t in range(NT):
                    rt = fp.tile([P, D], FP32, tag="res", bufs=NT,
                                 name=f"res{t}")
                    nc.sync.dma_start(rt, q_d[t * P:(t + 1) * P, :])
                    res.append(rt)

                # ---------------- attention ----------------
                with (
                    tc.tile_pool(name="att", bufs=1) as ap,
                    tc.tile_pool(name="psS", bufs=2, space="PSUM") as psS,
                    tc.tile_pool(name="psO", bufs=3, space="PSUM") as psO,
                    tc.tile_pool(name="psB", bufs=1, space="PSUM") as psB,
                ):
                    rc = RECIP_APPROX_FAST_CONSTS

                    def emit_pv(h, PT):
                        """PV matmuls for head h (consumes PT)."""
                        pos = []
                        for n in range(NQ):
                            po = psO.tile([E, QH], FP32, tag="o",
                                          name="ps_o")
                            pos.append(po)
                        for i in range(NT):
                            for n in range(NQ):
                                nc.tensor.matmul(
                                    pos[n],
                                    Vaug[:, i, h * E:(h + 1) * E],
                                    PT[:, i, n * QH:(n + 1) * QH],
                                    start=(i == 0), stop=(i == NT - 1))
                        return pos

                    def emit_epilogue(h, pos):
                        """Normalize O_h by sumexp and write OT.

                        The approx-fast DVE reciprocal only works at
                        partition 0, so the sumexp row goes psum(row 64)
                        -> sbuf(lane 64) -> DMA -> sbuf(lane 0).
                        """
                        c, half = h // 2, h % 2
                        rin = ap.tile([E, L], FP32, tag="rin", bufs=2,
                                      name="rin")
                        rl0 = ap.tile([1, L], FP32, tag="rl0", bufs=2,
                                      name="rl0")
                        rec = ap.tile([1, L], FP32R, tag="rec", bufs=2,
                                      name="rec")
                        otmp = None
                        if half == 1:
                            otmp = ap.tile([HD, L], BF16, tag="otmp",
                                           bufs=2, name="otmp")
                        for n in range(NQ):
                            nc.vector.tensor_copy(
                                rin[HD:E, n * QH:(n + 1) * QH],
                                pos[n][HD:E, :])
                        nc.sync.dma_start(rl0, rin[HD:E, :])
                        nc.vector._custom_dve(
                            RECIPROCAL_APPROX_FAST, out=rec, in0=rl0,
                            s0=rc["s0"], s1=rc["s1"], imm2=rc["imm2"])
                        for n in range(NQ):
                            ns = slice(n * QH, (n + 1) * QH)
                            po = pos[n]
                            pb = psB.tile([HD, QH], FP32, tag="b",
                                          name="ps_b")
                            nc.tensor.matmul(pb, ones64[0:1, :],
                                             rec[0:1, ns],
                                             start=True, stop=True)
                            pbs = ap.tile([HD, QH], FP32, tag="pbs",
                                          bufs=2, name="pbs")
                            nc.vector.tensor_copy(pbs, pb)
                            if half == 0:
                                nc.vector.tensor_tensor(
                                    OT[c][0:HD, ns], po[0:HD, :], pbs,
                                    OP.mult)
                            else:
                                nc.vector.tensor_tensor(
                                    otmp[:, ns], po[0:HD, :], pbs, OP.mult)
                        if half == 1:
                            nc.sync.dma_start(OT[c][HD:P, :], otmp)

                    prev = None  # (h, PT) pending PV
                    for h in range(H):
                        c, half = h // 2, h % 2
                        hs = slice(half * HD, half * HD + HD)
                        PT = ap.tile([P, NC, L], BF16, tag="pt", bufs=2,
                                     name="pt")
                        # S matmuls + exp for head h, with PV of head h-1
                        # interleaved chunk-by-chunk to keep the PE dense
                        pv_pos = None
                        if prev is not None:
                            ph, pPT = prev
                            pv_pos = [psO.tile([E, QH], FP32, tag="o",
                                               name="ps_o")
                                      for _ in range(NQ)]
                        for i in range(NT):
                            ks = slice(i * P, (i + 1) * P)
                            ss = psS.tile([P, L], FP32, tag="s", name="ps_s")
                            for n in range(NQ):
                                nc.tensor.matmul(
                                    ss[:, n * QH:(n + 1) * QH],
                                    KT[hs, c, ks],
                                    QT[hs, c, n * QH:(n + 1) * QH],
                                    start=True, stop=True)
                            nc.scalar.activation(PT[:, i, :], ss, AF.Exp)
                            if prev is not None:
                                for n in range(NQ):
                                    nc.tensor.matmul(
                                        pv_pos[n],
                                        Vaug[:, i, ph * E:(ph + 1) * E],
                                        pPT[:, i, n * QH:(n + 1) * QH],
                                        start=(i == 0), stop=(i == NT - 1))
                        if prev is not None:
                            emit_epilogue(prev[0], pv_pos)
                        prev = (h, PT)
                    # drain: PV + epilogue of the last head
                    pos = emit_pv(prev[0], prev[1])
                    emit_epilogue(prev[0], pos)

                # ------------- output projection + residual + LN ---------
                with tc.tile_pool(name="psW", bufs=2, space="PSUM") as psW:
                    for t in range(NT):
                        ps = psW.tile([P, D], FP32, tag="w", name="ps_w")
                        for n in range(NQ):
                            for j in range(NC):
                                nc.tensor.matmul(
                                    ps[:, n * QH:(n + 1) * QH],
                                    OT[j][:, t * P:(t + 1) * P],
                                    wo_t[j][:, n * QH:(n + 1) * QH],
                                    start=(j == 0), stop=(j == NC - 1))
                        u = fp.tile([P, D], FP32, tag="u", bufs=2, name="u")
                        nc.vector.tensor_tensor(u, ps, res[t], OP.add)
                        y = fp.tile([P, D], FP32, tag="y", bufs=2, name="y")
                        rstd, nmr = _emit_ln_stats(nc, fp, u, y, eps_t)
                        nc.scalar.activation(y, u, AF.Identity, bias=nmr,
                                             scale=rstd)
                        nc.vector.tensor_tensor(y, y, gamma_bc, OP.mult)
                        nc.vector.tensor_tensor(y, y, beta_bc, OP.add)
                        nc.sync.dma_start(out_d[t * P:(t + 1) * P, :], y)

    nc.compile()
    return nc


_CACHE = {}


def _get_nc():
    if "nc" not in _CACHE:
        _CACHE["nc"] = build_bass()
    return _CACHE["nc"]


def make_in_maps(q, k, v, Wq, Wk, Wv, Wo, gamma, beta):
    q = np.asarray(q, np.float32)
    kb = np.asarray(k, np.float32).astype(ml_dtypes.bfloat16)
    vb = np.asarray(v, np.float32).astype(ml_dtypes.bfloat16)
    gamma = np.asarray(gamma, np.float32)
    beta = np.asarray(beta, np.float32)
    Wq = np.asarray(Wq, np.float32)
    # fold pre-LN gamma/beta and the 1/sqrt(dk)=0.125 scale into Wq
    wq = (0.125 * gamma[:, None] * Wq).astype(ml_dtypes.bfloat16)
    bq = (0.125 * (beta @ Wq)).astype(np.float32)           # [D]
    bq_t = np.ascontiguousarray(bq.reshape(NC, P).T)        # [P, NC]
    wk = np.asarray(Wk, np.float32).astype(ml_dtypes.bfloat16)
    wv = np.asarray(Wv, np.float32).astype(ml_dtypes.bfloat16)
    wo = np.asarray(Wo, np.float32).astype(ml_dtypes.bfloat16)
    gb = np.ascontiguousarray(np.tile(gamma[None, :], (P, 1)))
    bb = np.ascontiguousarray(np.tile(beta[None, :], (P, 1)))
    ident = np.eye(P, dtype=np.float32).astype(ml_dtypes.bfloat16)
    ones64 = np.ones((P, HD), np.float32)
    epsc = np.full((P, 1), EPS, np.float32)
    vone = np.ones((P, H * E), ml_dtypes.bfloat16)
    B = q.shape[0]
    return [
        {
            "q": np.ascontiguousarray(q[b]),
            "kb": np.ascontiguousarray(kb[b]),
            "vb": np.ascontiguousarray(vb[b]),
            "wq": wq, "wk": wk, "wv": wv, "wo": wo, "bq": bq_t,
            "gb": gb, "bb": bb, "ident": ident, "ones64": ones64,
            "epsc": epsc, "vone": vone,
        }
        for b in range(B)
    ]


def kernel(q, k, v, Wq, Wk, Wv, Wo, gamma, beta, trace=False):
    from concourse.bass_utils import run_bass_kernel_spmd

    nc = _get_nc()
    in_maps = make_in_maps(q, k, v, Wq, Wk, Wv, Wo, gamma, beta)
    res = run_bass_kernel_spmd(nc, in_maps, core_ids=list(range(len(in_maps))),
                               trace=trace)
    out = np.stack([r["out"] for r in res.results], axis=0)
    if trace:
        return out, res
    return out


# revision 21
# speedup vs baseline: 1.3218x; 1.0831x over previous
"""Trainium2 Bass kernel: pre-LN multi-head attention block (B=8, L=1024,
D=1024, H=16, dk=dv=64), data-parallel over batch across 8 NeuronCores.

v2 design (all-bf16 PE path):
  - k, v pre-cast to bf16 on host; q stays fp32 (LN + residual precision).
  - pre-LN gamma/beta folded into Wq on host: Q = xhat @ wq' + bq, with the
    bias added during the PSUM->SBUF evacuation (per-partition AP scalar).
  - x^T built by PE transposes in bf16 (1 cyc/row), 8 blocks per psum tile,
    single strided DVE evac per token tile.
  - S^T per head via 64-partition-offset matmul operands (no zero-padding).
  - exp on ACT reads a [128,1024] psum tile (2 banks / 2 matmul groups).
  - PV with ones-augmented V (sumexp rides along as psum row 64); lane-64
    reciprocal + 1-partition matmul broadcast; no sumexp DMAs.
  - output projection flipped token-major (stationary O^T chunks, moving Wo
    rows): no output transposes; residual add + LN + store pipelined per
    token tile.
"""

import numpy as np
import ml_dtypes

import concourse.bass as bass
import concourse.mybir as mybir
import concourse.tile as tile
from concourse import bacc
from concourse.dve_ops import RECIP_APPROX_FAST_CONSTS, RECIPROCAL_APPROX_FAST

P = 128
L = 1024          # tokens per batch element
D = 1024          # model dim
H = 16            # heads
HD = 64           # head dim
E = HD + 1        # head dim + sumexp column
NC = D // P       # 8 feature chunks
NT = L // P       # 8 token chunks
NQ = 2            # 512-wide halves of the moving/free dimension
QH = 512
EPS = 1e-6

FP32 = mybir.dt.float32
BF16 = mybir.dt.bfloat16
FP32R = mybir.dt.float32r
OP = mybir.AluOpType
AF = mybir.ActivationFunctionType


def _emit_ln_stats(nc, pool, x, scratch, eps_t):
    """Return (rstd, neg_mu_rstd) per-partition [P,1] APs for LN of x."""
    st = pool.tile([P, 8], FP32, tag="lnst", bufs=4, name="lnst")
    nc.scalar.activation(scratch, x, AF.Copy, accum_out=st[:, 0:1])
    nc.scalar.activation(scratch, x, AF.Square, accum_out=st[:, 1:2])
    nc.vector.tensor_scalar_mul(st[:, 2:3], st[:, 0:1], 1.0 / D)     # mu
    nc.vector.tensor_tensor(st[:, 3:4], st[:, 2:3], st[:, 2:3], OP.mult)
    nc.vector.tensor_scalar_mul(st[:, 4:5], st[:, 1:2], 1.0 / D)     # E[x^2]
    nc.vector.tensor_tensor(st[:, 4:5], st[:, 4:5], st[:, 3:4], OP.subtract)
    nc.scalar.activation(st[:, 5:6], st[:, 4:5], AF.Sqrt, bias=eps_t)
    nc.vector.reciprocal(st[:, 6:7], st[:, 5:6])                     # rstd
    nc.vector.tensor_tensor(st[:, 7:8], st[:, 2:3], st[:, 6:7], OP.mult)
    nc.vector.tensor_scalar_mul(st[:, 7:8], st[:, 7:8], -1.0)        # -mu*rstd
    return st[:, 6:7], st[:, 7:8]


def build_bass():
    nc = bacc.Bacc("TRN2", target_bir_lowering=False, debug=False)

    q_d = nc.dram_tensor("q", [L, D], FP32, kind="ExternalInput")
    kb_d = nc.dram_tensor("kb", [L, D], BF16, kind="ExternalInput")
    vb_d = nc.dram_tensor("vb", [L, D], BF16, kind="ExternalInput")
    wq_d = nc.dram_tensor("wq", [D, D], BF16, kind="ExternalInput")
    wk_d = nc.dram_tensor("wk", [D, D], BF16, kind="ExternalInput")
    wv_d = nc.dram_tensor("wv", [D, D], BF16, kind="ExternalInput")
    wo_d = nc.dram_tensor("wo", [D, D], BF16, kind="ExternalInput")
    bq_d = nc.dram_tensor("bq", [P, NC], FP32, kind="ExternalInput")
    gb_d = nc.dram_tensor("gb", [P, D], FP32, kind="ExternalInput")
    bb_d = nc.dram_tensor("bb", [P, D], FP32, kind="ExternalInput")
    id_d = nc.dram_tensor("ident", [P, P], BF16, kind="ExternalInput")
    on_d = nc.dram_tensor("ones64", [P, HD], FP32R, kind="ExternalInput")
    ep_d = nc.dram_tensor("epsc", [P, 1], FP32, kind="ExternalInput")
    vo_d = nc.dram_tensor("vone", [P, H * E], BF16, kind="ExternalInput")
    out_d = nc.dram_tensor("out", [L, D], FP32, kind="ExternalOutput")

    with tile.TileContext(nc) as tc:
        with tc.tile_pool(name="persist", bufs=1) as pp:
            ident = pp.tile([P, P], BF16, name="ident")
            eps_t = pp.tile([P, 1], FP32, name="eps_t")
            ones64 = pp.tile([P, HD], FP32R, name="ones64")
            bq_t = pp.tile([P, NC], FP32, name="bq_t")
            KT = pp.tile([P, NC, L], BF16, name="KT")
            QT = pp.tile([P, NC, L], BF16, name="QT")
            Vaug = pp.tile([P, NT, H * E], BF16, name="Vaug")
            OT = [pp.tile([P, L], BF16, name=f"ot{j}") for j in range(H // 2)]

            nc.sync.dma_start(ident, id_d[:])
            nc.sync.dma_start(eps_t, ep_d[:])
            nc.sync.dma_start(ones64, on_d[:])
            nc.sync.dma_start(bq_t, bq_d[:])

            # ---------------- QKV phase ----------------
            with (
                tc.tile_pool(name="qkv", bufs=1) as qp,
                tc.tile_pool(name="psA", bufs=1, space="PSUM") as psA,
            ):
                def load_w(dram, nm):
                    tiles = []
                    for i in range(NC):
                        wt = qp.tile([P, D], BF16, tag=f"w{nm}", bufs=NC,
                                     name=f"w{nm}{i}")
                        nc.sync.dma_start(wt, dram[i * P:(i + 1) * P, :])
                        tiles.append(wt)
                    return tiles

                def transpose_tile(dst, x, t):
                    """dst[:, c, t*128:+128] = x[:, c*128:+128]^T for all c."""
                    pt = psA.tile([P, D], BF16, tag="tr", bufs=2, name="ps_tr")
                    for c in range(NC):
                        nc.tensor.transpose(
                            pt[:, c * P:(c + 1) * P],
                            x[:, c * P:(c + 1) * P], ident)
                    nc.vector.tensor_copy(
                        dst[:, :, t * P:(t + 1) * P],
                        pt.rearrange("p (c x) -> p c x", x=P))

                def xT_tile():
                    return qp.tile([P, NC, L], BF16, tag="xT", bufs=2,
                                   name="xT")

                # ---- k -> kT -> K-proj ----
                # kb DMAs go first on the sync queue; tile 0 is split into
                # column chunks across DMA queues so the first transpose can
                # start within ~2us.
                kT = xT_tile()
                kin = []
                for t in range(NT):
                    x = qp.tile([P, D], BF16, tag="kin", bufs=NT, name="k_in")
                    if t == 0:
                        for c in range(NC):
                            cs = slice(c * P, (c + 1) * P)
                            nc.sync.dma_start(x[:, cs], kb_d[0:P, cs])
                    else:
                        nc.sync.dma_start(x, kb_d[t * P:(t + 1) * P, :])
                    kin.append(x)
                wk_t = load_w(wk_d, "k")

                # q DMA + LN emitted early: the ACT-side LN overlaps the
                # k transposes / K-proj so the qn transposes are not gated
                qn = []
                for t in range(NT):
                    x = qp.tile([P, D], FP32, tag="qin", bufs=3, name="q_in")
                    nc.sync.dma_start(x, q_d[t * P:(t + 1) * P, :])
                    y = qp.tile([P, D], BF16, tag="qn", bufs=NT, name="qn")
                    rstd, nmr = _emit_ln_stats(nc, qp, x, y, eps_t)
                    nc.scalar.activation(y, x, AF.Identity, bias=nmr,
                                         scale=rstd)
                    qn.append(y)

                for t in range(NT):
                    transpose_tile(kT, kin[t], t)

                def proj_feat(w_tiles, src, dst, bias_col=None):
                    for m in range(NC):
                        ps = psA.tile([P, L], FP32, tag="pj", bufs=2,
                                      name="ps_pj")
                        for n in range(NQ):
                            for i in range(NC):
                                nc.tensor.matmul(
                                    ps[:, n * QH:(n + 1) * QH],
                                    w_tiles[i][:, m * P:(m + 1) * P],
                                    src[:, i, n * QH:(n + 1) * QH],
                                    start=(i == 0), stop=(i == NC - 1))
                        if bias_col is None:
                            nc.vector.tensor_copy(dst[:, m, :], ps)
                        else:
                            nc.vector.tensor_scalar_add(
                                dst[:, m, :], ps, bias_col[:, m:m + 1])

                # prefetch v tiles + ones columns + Wv while K-proj runs
                vin = []
                for t in range(NT):
                    x = qp.tile([P, D], BF16, tag="vin", bufs=NT, name="v_in")
                    nc.sync.dma_start(x, vb_d[t * P:(t + 1) * P, :])
                    vin.append(x)
                for t in range(NT):
                    nc.sync.dma_start(Vaug[:, t, :], vo_d[:])
                wv_t = load_w(wv_d, "v")

                proj_feat(wk_t, kT, KT)

                # ---- v -> vT -> V-proj (token-major, into Vaug) ----
                vT = xT_tile()
                for t in range(NT):
                    transpose_tile(vT, vin[t], t)
                wq_t = load_w(wq_d, "q")
                for t in range(NT):
                    ps = psA.tile([P, L], FP32, tag="pj", bufs=2, name="ps_v")
                    for n in range(NQ):
                        for i in range(NC):
                            nc.tensor.matmul(
                                ps[:, n * QH:(n + 1) * QH],
                                vT[:, i, t * P:(t + 1) * P],
                                wv_t[i][:, n * QH:(n + 1) * QH],
                                start=(i == 0), stop=(i == NC - 1))
                    dst = Vaug[:, t, :].rearrange("p (h e) -> p h e", e=E)
                    nc.vector.tensor_copy(
                        dst[:, :, 0:HD],
                        ps.rearrange("p (h x) -> p h x", x=HD))

                # ---- qn -> qnT -> Q-proj (bias folded) ----
                qnT = xT_tile()
                for t in range(NT):
                    transpose_tile(qnT, qn[t], t)
                proj_feat(wq_t, qnT, QT, bias_col=bq_t)

            # ---------------- out-phase inputs (emit DMAs early) ----------
            with tc.tile_pool(name="fin", bufs=1) as fp:
                gamma_bc = fp.tile([P, D], FP32, name="gamma_bc")
                beta_bc = fp.tile([P, D], FP32, name="beta_bc")
                nc.sync.dma_start(gamma_bc, gb_d[:])
                nc.sync.dma_start(beta_bc, bb_d[:])
                wo_t = []
                for j in range(NC):
                    wt = fp.tile([P, D], BF16, tag="wo", bufs=NC,
                                 name=f"wo{j}")
                    nc.sync.dma_start(wt, wo_d[j * P:(j + 1) * P, :])
                    wo_t.append(wt)
                res = []
                for t in range(NT):
                    rt = fp.tile([P, D], FP32, tag="res", bufs=3,
                                 name=f"res{t}")
                    nc.sync.dma_start(rt, q_d[t * P:(t + 1) * P, :])
                    res.append(rt)

                # ---------------- attention ----------------
                with (
                    tc.tile_pool(name="att", bufs=1) as ap,
                    tc.tile_pool(name="psS", bufs=2, space="PSUM") as psS,
                    tc.tile_pool(name="psO", bufs=3, space="PSUM") as psO,
                    tc.tile_pool(name="psB", bufs=1, space="PSUM") as psB,
                ):
                    rc = RECIP_APPROX_FAST_CONSTS

                    def emit_pv(h, PT):
                        """PV matmuls for head h (consumes PT)."""
                        pos = []
                        for n in range(NQ):
                            po = psO.tile([E, QH], FP32, tag="o",
                                          name="ps_o")
                            pos.append(po)
                        for i in range(NT):
                            for n in range(NQ):
                                nc.tensor.matmul(
                                    pos[n],
                                    Vaug[:, i, h * E:(h + 1) * E],
                                    PT[:, i, n * QH:(n + 1) * QH],
                                    start=(i == 0), stop=(i == NT - 1))
                        return pos

                    def emit_epilogue(h, pos):
                        """Normalize O_h by sumexp and write OT.

                        The approx-fast DVE reciprocal only works at
                        partition 0, so the sumexp row goes psum(row 64)
                        -> sbuf(lane 64) -> DMA -> sbuf(lane 0).
                        """
                        c, half = h // 2, h % 2
                        rin = ap.tile([E, L], FP32, tag="rin", bufs=2,
                                      name="rin")
                        rec = ap.tile([1, L], FP32R, tag="rec", bufs=2,
                                      name="rec")
                        otmp = None
                        if half == 1:
                            otmp = ap.tile([HD, L], BF16, tag="otmp",
                                           bufs=2, name="otmp")
                        for n in range(NQ):
                            nc.vector.tensor_copy(
                                rin[HD:E, n * QH:(n + 1) * QH],
                                pos[n][HD:E, :])
                        nc.gpsimd.dma_start(rin[0:1, :], rin[HD:E, :])
                        nc.vector._custom_dve(
                            RECIPROCAL_APPROX_FAST, out=rec, in0=rin[0:1, :],
                            s0=rc["s0"], s1=rc["s1"], imm2=rc["imm2"])
                        for n in range(NQ):
                            ns = slice(n * QH, (n + 1) * QH)
                            po = pos[n]
                            pb = psB.tile([HD, QH], FP32, tag="b",
                                          name="ps_b")
                            nc.tensor.matmul(pb, ones64[0:1, :],
                                             rec[0:1, ns],
                                             start=True, stop=True)
                            pbs = ap.tile([HD, QH], FP32, tag="pbs",
                                          bufs=2, name="pbs")
                            nc.vector.tensor_copy(pbs, pb)
                            if half == 0:
                                nc.vector.tensor_tensor(
                                    OT[c][0:HD, ns], po[0:HD, :], pbs,
                                    OP.mult)
                            else:
                                nc.vector.tensor_tensor(
                                    otmp[:, ns], po[0:HD, :], pbs, OP.mult)
                        if half == 1:
                            nc.gpsimd.dma_start(OT[c][HD:P, :], otmp)

                    prev = None  # (h, PT) pending PV
                    for h in range(H):
                        c, half = h // 2, h % 2
                        hs = slice(half * HD, half * HD + HD)
                        PT = ap.tile([P, NC, L], BF16, tag="pt", bufs=2,
                                     name="pt")
                        # S matmuls + exp for head h, with PV of head h-1
                        # interleaved chunk-by-chunk to keep the PE dense
                        pv_pos = None
                        if prev is not None:
                            ph, pPT = prev
                            pv_pos = [psO.tile([E, QH], FP32, tag="o",
                                               name="ps_o")
                                      for _ in range(NQ)]
                        for i in range(NT):
                            ks = slice(i * P, (i + 1) * P)
                            ss = psS.tile([P, L], FP32, tag="s", name="ps_s")
                            for n in range(NQ):
                                nc.tensor.matmul(
                                    ss[:, n * QH:(n + 1) * QH],
                                    KT[hs, c, ks],
                                    QT[hs, c, n * QH:(n + 1) * QH],
                                    start=True, stop=True)
                            nc.scalar.activation(PT[:, i, :], ss, AF.Exp)
                            if prev is not None:
                                for n in range(NQ):
                                    nc.tensor.matmul(
                                        pv_pos[n],
                                        Vaug[:, i, ph * E:(ph + 1) * E],
                                        pPT[:, i, n * QH:(n + 1) * QH],
                                        start=(i == 0), stop=(i == NT - 1))
                        if prev is not None:
                            emit_epilogue(prev[0], pv_pos)
                        prev = (h, PT)
                    # drain: PV + epilogue of the last head
                    pos = emit_pv(prev[0], prev[1])
                    emit_epilogue(prev[0], pos)

                # ------------- output projection + residual + LN ---------
                # LN stats via bn_stats (DVE); apply split across GpSimd
                # and DVE STT ops: y = ((u - mu) * gamma) * rstd + beta
                with tc.tile_pool(name="psW", bufs=3, space="PSUM") as psW:
                    for t in range(NT):
                        ps = psW.tile([P, D], FP32, tag="w", name="ps_w")
                        for n in range(NQ):
                            for j in range(NC):
                                nc.tensor.matmul(
                                    ps[:, n * QH:(n + 1) * QH],
                                    OT[j][:, t * P:(t + 1) * P],
                                    wo_t[j][:, n * QH:(n + 1) * QH],
                                    start=(j == 0), stop=(j == NC - 1))
                        u = fp.tile([P, D], FP32, tag="u", bufs=2, name="u")
                        nc.vector.tensor_tensor(u, ps, res[t], OP.add)
                        stt = fp.tile([P, 12], FP32, tag="stt", bufs=2,
                                      name="stt")
                        nc.vector.bn_stats(stt[:, 0:6], u[:, 0:QH])
                        nc.vector.bn_stats(stt[:, 6:12], u[:, QH:D])
                        mv = fp.tile([P, 6], FP32, tag="mv", bufs=2,
                                     name="mv")
                        nc.vector.bn_aggr(mv[:, 0:2], stt)
                        nc.scalar.activation(mv[:, 2:3], mv[:, 1:2],
                                             AF.Sqrt, bias=eps_t)
                        nc.vector.reciprocal(mv[:, 3:4], mv[:, 2:3])
                        nc.vector.tensor_tensor(mv[:, 4:5], mv[:, 0:1],
                                                mv[:, 3:4], OP.mult)
                        nc.vector.tensor_scalar_mul(mv[:, 4:5], mv[:, 4:5],
                                                    -1.0)
                        t1 = fp.tile([P, D], FP32, tag="t1", bufs=2,
                                     name="t1")
                        nc.scalar.activation(t1, u, AF.Identity,
                                             bias=mv[:, 4:5],
                                             scale=mv[:, 3:4])
                        t2 = fp.tile([P, D], FP32, tag="t2", bufs=2,
                                     name="t2")
                        nc.gpsimd.tensor_tensor(t2, t1, gamma_bc, OP.mult)
                        y = fp.tile([P, D], FP32, tag="y", bufs=2, name="y")
                        nc.vector.tensor_tensor(y, t2, beta_bc, OP.add)
                        nc.sync.dma_start(out_d[t * P:(t + 1) * P, :], y)

    nc.compile()
    return nc


_CACHE = {}


def _get_nc():
    if "nc" not in _CACHE:
        _CACHE["nc"] = build_bass()
    return _CACHE["nc"]


def make_in_maps(q, k, v, Wq, Wk, Wv, Wo, gamma, beta):
    q = np.asarray(q, np.float32)
    kb = np.asarray(k, np.float32).astype(ml_dtypes.bfloat16)
    vb = np.asarray(v, np.float32).astype(ml_dtypes.bfloat16)
    gamma = np.asarray(gamma, np.float32)
    beta = np.asarray(beta, np.float32)
    Wq = np.asarray(Wq, np.float32)
    # fold pre-LN gamma/beta and the 1/sqrt(dk)=0.125 scale into Wq
    wq = (0.125 * gamma[:, None] * Wq).astype(ml_dtypes.bfloat16)
    bq = (0.125 * (beta @ Wq)).astype(np.float32)           # [D]
    bq_t = np.ascontiguousarray(bq.reshape(NC, P).T)        # [P, NC]
    wk = np.asarray(Wk, np.float32).astype(ml_dtypes.bfloat16)
    wv = np.asarray(Wv, np.float32).astype(ml_dtypes.bfloat16)
    wo = np.asarray(Wo, np.float32).astype(ml_dtypes.bfloat16)
    gb = np.ascontiguousarray(np.tile(gamma[None, :], (P, 1)))
    bb = np.ascontiguousarray(np.tile(beta[None, :], (P, 1)))
    ident = np.eye(P, dtype=np.float32).astype(ml_dtypes.bfloat16)
    ones64 = np.ones((P, HD), np.float32)
    epsc = np.full((P, 1), EPS, np.float32)
    vone = np.ones((P, H * E), ml_dtypes.bfloat16)
    B = q.shape[0]
    return [
        {
            "q": np.ascontiguousarray(q[b]),
            "kb": np.ascontiguousarray(kb[b]),
            "vb": np.ascontiguousarray(vb[b]),
            "wq": wq, "wk": wk, "wv": wv, "wo": wo, "bq": bq_t,
            "gb": gb, "bb": bb, "ident": ident, "ones64": ones64,
            "epsc": epsc, "vone": vone,
        }
        for b in range(B)
    ]


def kernel(q, k, v, Wq, Wk, Wv, Wo, gamma, beta, trace=False):
    from concourse.bass_utils import run_bass_kernel_spmd

    nc = _get_nc()
    in_maps = make_in_maps(q, k, v, Wq, Wk, Wv, Wo, gamma, beta)
    res = run_bass_kernel_spmd(nc, in_maps, core_ids=list(range(len(in_maps))),
                               trace=trace)
    out = np.stack([r["out"] for r in res.results], axis=0)
    if trace:
        return out, res
    return out
